# revision 52
# baseline (speedup 1.0000x reference)
"""Trainium2 Bass kernel for nn_Block_9534827397286 (sparse_attention decode).

Single-token paged-attention decode block:
  qkv = x @ Wqkv.T; quantize new k/v (per-tensor int8) into page cache;
  dequant + attention over 8192 cached tokens; out proj + residual.

Sharding (8 cores): head-parallel. Core m owns heads 4m..4m+3, the matching
row-slices of Wqkv, column-slices of Wproj, and its heads' K/V cache pages.
The single global quantization scale (max |k| over ALL heads) is computed
with a tiny in-kernel AllReduce(max); the output projection partial sums are
reduced on the host during unshard.

The kernel is HBM-DMA-bound (358 GB/s per core), so everything shipped is
8-bit: weights are pre-cast to fp8e4m3 on the host, and the int8 KV cache is
dequantized *with its per-page scales folded in* (K also folds 1/sqrt(dh))
straight to fp8 — per-core traffic drops from 24.3 MiB (fp16 weights +
int8 cache + expanded scale tiles) to 16.1 MiB, and the on-chip int8->fp16
convert traffic (~25 us DVE + ~30 us ACT in the baseline) disappears:
scores come out of the PE pre-scaled, exp reads PSUM directly and emits
fp8 attention weights for the aV matmuls.

Numerics: the block output is x + proj(attn); the attn contribution is tiny
relative to the residual, so ~5% fp8 relative error lands ~1e-3 rel overall
(budget 2e-2). Ranges: main-chunk attn weights exp(~N(0,0.4)) in [0.2, 5.5]
fit e4m3; the last 16 positions are rescaled by the new token's scale and
can reach exp(~9)*vsc ~ 400 > e4m3 max, so the whole last l-chunk of V runs
in fp16 (vtl + attn column 63).

The last page's scale overwrite + new-token insert (AllReduce-gated) uses
separate last-chunk tiles so chunks 0..62 never wait on the collective; the
new token's V contribution is a rank-1 correction matmul.

Bulk DMA order (sync ring, program order): x -> last-chunk tiles + scales
(tiny) -> Wkv FIRST (it feeds the AllReduce; its result gates per-head
combines, so the round-trip must resolve early) -> Wq -> head-0 K/V ->
head-1 K/V -> Wproj -> head-2 K/V -> head-3 K/V quarter-interleaved so the
tail compute chases the last DMA. The collective's cc_in rides the scalar
HWDGE ring and gmax returns via SWDGE — neither blocks the bulk sync-ring
FIFO nor the ACT ring that issues the exps.
"""

import math

import numpy as np
import ml_dtypes

import concourse.bass as bass
import concourse.mybir as mybir
import concourse.tile as tile
from concourse import bacc
from concourse.bass_utils import run_bass_kernel_spmd

# Problem constants (hardcoded per contract; kernel.py must be self-contained)
D_MODEL = 4096
NUM_HEADS = 32
HEAD_DIM = 128
PAGE_SIZE = 16
PAGES_USED = 512
KV_LEN = PAGES_USED * PAGE_SIZE  # 8192
N_CORES = 8
H_LOC = NUM_HEADS // N_CORES  # 4 heads per core
N_CHUNKS = KV_LEN // 128  # 64 l-chunks of the attention
N_MAIN = N_CHUNKS - 1  # chunks 0..62: host-folded scales
L_MAIN = 128 * N_MAIN  # 8064
N_CI = D_MODEL // 128  # 32 contraction chunks for the qkv matvec

F16 = mybir.dt.float16
F32 = mybir.dt.float32
F8 = mybir.dt.float8e4
NP_F8 = ml_dtypes.float8_e4m3

INV_SQRT_DH = 1.0 / math.sqrt(HEAD_DIM)
# score/exp/aV pipeline pieces (chunk ranges); finer near the end so the
# head-3 tail chases its piece-interleaved DMA closely
PIECES = [(0, 16), (16, 32), (32, 48), (48, 63)]
# fp8 range-centering for the per-head attn output. |attn out| is bounded by
# max|V| ~ 127*vsc_new ~ 4.6 (the rescaled last page can dominate the
# softmax), so 32 keeps the worst case ~147 < 240 (e4m3 max) while lifting
# typical values (~0.005) well into the normal range.
A_SCALE = 32.0
DR = mybir.MatmulPerfMode.DoubleRow


def build_bass(n_iter: int = 1, with_collective: bool = True, debug_out: bool = False):
    """Build the SPMD Bass program (identical on all 8 cores).

    n_iter > 1 unrolls the whole body N times (timing harness only).
    with_collective=False replaces the AllReduce with a local DMA copy
    (TimelineSim can't model collectives; timing harness only).
    """
    nc = bacc.Bacc("TRN2", num_devices=N_CORES)

    # Per-core inputs (host ships per-core slices in SBUF-friendly layouts).
    # Weights are shipped as fp8(64*W): their raw values (~N(0, 1/4096))
    # sit at e4m3's subnormal boundary; x64 recenters them into the normal
    # range (the 64 is divided back out of q and the k/v quant scales).
    xk8_d = nc.dram_tensor("xk8", [128, 512], F8, kind="ExternalInput")
    wq_d = nc.dram_tensor("wq", [128, N_CI, 512], F8, kind="ExternalInput")
    wkv_d = nc.dram_tensor("wkv", [128, N_CI, 1024], F8, kind="ExternalInput")
    wp_d = nc.dram_tensor("wp", [H_LOC, 128, D_MODEL], F8, kind="ExternalInput")
    kt_d = nc.dram_tensor("kt", [H_LOC, 128, L_MAIN], F8, kind="ExternalInput")
    vt_d = nc.dram_tensor("vt", [H_LOC, 128, L_MAIN], F8, kind="ExternalInput")
    # fp16 pack: [0:512] = V last chunk [l, 128h+d]; [512:1024] = raw-int8 K
    # last chunk [d, 512+128h+j] — the rescaled tail runs entirely in fp16
    # (winner-take-most softmax there is sensitive to per-element noise)
    vtl_d = nc.dram_tensor("vtl", [128, 2 * H_LOC * 128], F16, kind="ExternalInput")
    # last-chunk dequant scale columns: [p, 2h] = K (incl 1/sqrt(dh)), [p, 2h+1] = V
    kvs_d = nc.dram_tensor("kvs", [128, 2 * H_LOC], F32, kind="ExternalInput")
    out_d = nc.dram_tensor("out", [128, D_MODEL // 128], F32, kind="ExternalOutput")
    dbg_d = None
    if debug_out:
        dbg_d = nc.dram_tensor("dbg", [128, 64], F32, kind="ExternalOutput")
    cc_in = nc.dram_tensor("cc_in", [2], F32)
    cc_out = nc.dram_tensor("cc_out", [2], F32, addr_space="Shared")

    with tile.TileContext(nc) as tc:
      for _it in range(n_iter):
        with (
            tc.tile_pool(name="const", bufs=1) as cpool,
            tc.tile_pool(name="wts", bufs=1) as wpool,
            tc.tile_pool(name="kv8", bufs=3) as kv8pool,
            tc.tile_pool(name="small", bufs=2) as spool,
            tc.tile_pool(name="rows", bufs=1) as rpool,
            tc.tile_pool(name="attn", bufs=2) as apool,
        ):
            # ---- constants ----
            ones_row = cpool.tile([1, 128], F32, tag="ones_row")
            nc.vector.memset(ones_row[:], 1.0)
            r256_row = cpool.tile([1, 128], F32, tag="r256_row")
            nc.vector.memset(r256_row[:], A_SCALE)
            ones_col = cpool.tile([128, 1], F32, tag="ones_col")
            nc.vector.memset(ones_col[:], 1.0)
            one_1 = cpool.tile([1, 1], F32, tag="one_1")
            nc.vector.memset(one_1[:], 1.0)
            # mask: 1 on partitions 112..127 (page-511 rows of chunk 63)
            mask_tail = cpool.tile([128, 1], mybir.dt.int16, tag="mask_tail")
            nc.gpsimd.memset(mask_tail[:], 1)
            nc.gpsimd.affine_select(
                out=mask_tail[:], in_=mask_tail[:],
                compare_op=mybir.AluOpType.is_ge, fill=0,
                base=-112, pattern=[[0, 1]], channel_multiplier=1,
            )
            mask_127 = cpool.tile([128, 1], mybir.dt.int16, tag="mask_127")
            nc.gpsimd.memset(mask_127[:], 1)
            nc.gpsimd.affine_select(
                out=mask_127[:], in_=mask_127[:],
                compare_op=mybir.AluOpType.is_ge, fill=0,
                base=-127, pattern=[[0, 1]], channel_multiplier=1,
            )
            zero_col = cpool.tile([128, 1], F32, tag="zero_col")
            nc.vector.memset(zero_col[:], 0.0)

            # ---- early DMAs: x + the tiny AllReduce-gated last-chunk data ----
            xk8 = cpool.tile([128, 512], F8, tag="xk8")
            nc.sync.dma_start(xk8[:], xk8_d[:])
            x8 = xk8[:, 0:512].rearrange("p (j r) -> p j r", r=16)
            kvs = cpool.tile([128, 2 * H_LOC], F32, tag="kvs")
            nc.sync.dma_start(kvs[:], kvs_d[:])
            vtl_all = cpool.tile([128, 2 * H_LOC * 128], F16, tag="vtl_all")
            nc.sync.dma_start(vtl_all[:], vtl_d[:])
            vtl_tiles = [vtl_all[:, 128 * h : 128 * (h + 1)] for h in range(H_LOC)]
            ktl_tiles = [
                vtl_all[:, 512 + 128 * h : 512 + 128 * (h + 1)] for h in range(H_LOC)
            ]

            q8 = cpool.tile([128, H_LOC], F8, tag="q8")
            qcol16 = cpool.tile([128, H_LOC], F16, tag="qcol16")
            a8 = cpool.tile([128, H_LOC], F8, tag="a8")
            sb_bc = cpool.tile([128, 8], F32, tag="sb_bc")
            k_ins16 = cpool.tile([128, H_LOC], F16, tag="k_ins16")
            v_ins = cpool.tile([1, 512], F16, tag="v_ins")

            # ================= phase A: qkv matvec + quantization =============
            with tc.tile_pool(name="psA", bufs=1, space="PSUM") as psA:
                # --- k/v weights first: they feed the AllReduce, whose
                # round-trip must resolve before the bulk stream needs it ---
                wkv_t = wpool.tile([128, N_CI, 1024], F8, tag="wkv_t")
                nc.sync.dma_start(wkv_t[:], wkv_d[:])
                wq_t = wpool.tile([128, N_CI, 512], F8, tag="wq_t")
                nc.sync.dma_start(wq_t[:], wq_d[:])
                cache_tiles = []
                kt8 = kv8pool.tile([128, L_MAIN], F8, tag="kt8")
                nc.sync.dma_start(kt8[:], kt_d[0])
                vt8 = kv8pool.tile([128, L_MAIN], F8, tag="vt8")
                nc.sync.dma_start(vt8[:], vt_d[0])
                cache_tiles.append((kt8, vt8))

                ps_k = psA.tile([1, 512], F32, tag="ps_k")
                ps_v = psA.tile([1, 512], F32, tag="ps_v")
                for j in range(N_CI // 2):
                    st = dict(start=(j == 0), stop=(j == N_CI // 2 - 1))
                    nc.tensor.matmul(
                        ps_k[:], x8[:, 2 * j : 2 * j + 2, 0:1],
                        wkv_t[:, 2 * j : 2 * j + 2, 0:512], perf_mode=DR, **st,
                    )
                    nc.tensor.matmul(
                        ps_v[:], x8[:, 2 * j : 2 * j + 2, 0:1],
                        wkv_t[:, 2 * j : 2 * j + 2, 512:1024], perf_mode=DR, **st,
                    )

                # local |k|,|v| max -> AllReduce(max) across cores.
                # The tiny cc DMAs ride the scalar HWDGE ring so they never
                # block the bulk sync-ring FIFO.
                kvabs = spool.tile([1, 2], F32, tag="kvabs")
                nc.vector.reduce_max(
                    kvabs[:, 0:1], ps_k[:], axis=mybir.AxisListType.X,
                    apply_absolute_value=True,
                )
                nc.vector.reduce_max(
                    kvabs[:, 1:2], ps_v[:], axis=mybir.AxisListType.X,
                    apply_absolute_value=True,
                )
                nc.scalar.dma_start(cc_in[None, :], kvabs[:])
                if with_collective:
                    nc.gpsimd.collective_compute(
                        "AllReduce",
                        mybir.AluOpType.max,
                        replica_groups=[list(range(N_CORES))],
                        ins=[cc_in[:]],
                        outs=[cc_out[:]],
                    )
                # gmax returns on the GPSIMD (SWDGE) ring: its issue waits on
                # the collective, which would head-of-line block the ACT ring
                # (exp instructions) or the sync ring (bulk stream)
                gmax = spool.tile([1, 2], F32, tag="gmax")
                if with_collective:
                    nc.gpsimd.dma_start(gmax[:], cc_out[None, :])
                else:
                    # sim-only: same two-DMA-hop shape, skip the collective
                    nc.scalar.dma_start(gmax[:], cc_in[None, :])

                # --- q part: all scores depend on it ---
                ps_q = psA.tile([1, 512], F32, tag="ps_q")
                for j in range(N_CI // 2):
                    nc.tensor.matmul(
                        ps_q[:], x8[:, 2 * j : 2 * j + 2, 0:1],
                        wq_t[:, 2 * j : 2 * j + 2, :],
                        start=(j == 0), stop=(j == N_CI // 2 - 1), perf_mode=DR,
                    )
                q_rows = rpool.tile([1, 512], F32, tag="q_rows")
                nc.scalar.copy(out=q_rows[:], in_=ps_q[:])
                ps_trq = psA.tile([128, H_LOC], F32, tag="ps_trq")
                for h in range(H_LOC):
                    nc.tensor.matmul(
                        ps_trq[:, h : h + 1],
                        q_rows[:, 128 * h : 128 * (h + 1)], one_1[:],
                        start=True, stop=True,
                    )
                # ps_trq holds 64*q — divide the weight recentering back out
                nc.vector.tensor_scalar_mul(q8[:], ps_trq[:], 1.0 / 64.0)
                nc.scalar.activation(
                    qcol16[:], ps_trq[:],
                    mybir.ActivationFunctionType.Copy, scale=1.0 / 64.0,
                )

                # scales (the matvec emitted 64*k, 64*v because of the x64
                # weight recentering; gmax = 64*max|k|, 64*max|v|):
                # [ksc, vsc, 1/(64 ksc), 1/(64 vsc), ksc/sqrt(dh), 64ksc, 64vsc]
                scal = spool.tile([1, 8], F32, tag="scal")
                nc.vector.memset(scal[:], 0.0)
                nc.vector.tensor_scalar(
                    scal[:, 5:7], gmax[:, 0:2], 1.0 / 127.0, 64e-6,
                    op0=mybir.AluOpType.mult, op1=mybir.AluOpType.add,
                )
                nc.vector.tensor_scalar_mul(scal[:, 0:2], scal[:, 5:7], 1.0 / 64.0)
                nc.vector.reciprocal(scal[:, 2:3], scal[:, 5:6])
                nc.vector.reciprocal(scal[:, 3:4], scal[:, 6:7])
                nc.vector.tensor_scalar_mul(scal[:, 4:5], scal[:, 0:1], INV_SQRT_DH)

                # k/v psum rows -> SBUF
                kv_rows = rpool.tile([1, 1024], F32, tag="kv_rows")
                nc.scalar.copy(out=kv_rows[:, 0:512], in_=ps_k[:])
                nc.scalar.copy(out=kv_rows[:, 512:1024], in_=ps_v[:])

                # one transient bank: scale bcast (cols 0:8) + k cols (8:12)
                ps_tr = psA.tile([128, 12], F32, tag="ps_tr")
                nc.tensor.matmul(ps_tr[:, 0:8], ones_row[:], scal[:], start=True, stop=True)
                for h in range(H_LOC):
                    nc.tensor.matmul(
                        ps_tr[:, 8 + h : 9 + h],
                        kv_rows[:, 128 * h : 128 * (h + 1)], one_1[:],
                        start=True, stop=True,
                    )
                nc.vector.tensor_copy(out=sb_bc[:], in_=ps_tr[:, 0:8])

                # quantize new-token k (per-head cols): round(k/ksc)
                kq = spool.tile([128, H_LOC], F32, tag="kq")
                nc.vector.tensor_scalar_mul(kq[:], ps_tr[:, 8:12], sb_bc[:, 2:3])
                kmask = spool.tile([128, H_LOC], F32, tag="kmask")
                nc.vector.tensor_scalar(
                    kmask[:], kq[:], 0.0, -0.5,
                    op0=mybir.AluOpType.is_ge, op1=mybir.AluOpType.add,
                )  # +0.5 if >=0 else -0.5
                nc.vector.tensor_add(out=kq[:], in0=kq[:], in1=kmask[:])
                k_i8 = spool.tile([128, H_LOC], mybir.dt.int8, tag="k_i8")
                nc.vector.tensor_copy(out=k_i8[:], in_=kq[:])  # trunc toward 0
                nc.vector.tensor_copy(out=k_ins16[:], in_=k_i8[:])

                # quantize new-token v (row layout): round(v/vsc)
                vq = rpool.tile([1, 512], F32, tag="vq")
                nc.vector.tensor_scalar_mul(vq[:], kv_rows[:, 512:1024], scal[:, 3:4])
                vmask = rpool.tile([1, 512], F32, tag="vmask")
                nc.vector.tensor_scalar(
                    vmask[:], vq[:], 0.0, -0.5,
                    op0=mybir.AluOpType.is_ge, op1=mybir.AluOpType.add,
                )
                nc.vector.tensor_add(out=vq[:], in0=vq[:], in1=vmask[:])
                v_i8 = rpool.tile([1, 512], mybir.dt.int8, tag="v_i8")
                nc.vector.tensor_copy(out=v_i8[:], in_=vq[:])
                nc.vector.tensor_copy(out=v_ins[:], in_=v_i8[:])

                # --- cache DMAs for head 1, then proj weights, heads 2..3;
                # head 3 piece-interleaved so tail compute chases the DMA ---
                wp_tiles = []
                for h in range(1, H_LOC):
                    kt8 = kv8pool.tile([128, L_MAIN], F8, tag="kt8")
                    vt8 = kv8pool.tile([128, L_MAIN], F8, tag="vt8")
                    if h == H_LOC - 1:
                        for c0, c1 in ((0, 16), (16, 32), (32, 48), (48, 63)):
                            lo, hi = 128 * c0, 128 * c1
                            nc.sync.dma_start(kt8[:, lo:hi], kt_d[h][:, lo:hi])
                            nc.sync.dma_start(vt8[:, lo:hi], vt_d[h][:, lo:hi])
                    else:
                        nc.sync.dma_start(kt8[:], kt_d[h])
                        nc.sync.dma_start(vt8[:], vt_d[h])
                    cache_tiles.append((kt8, vt8))
                    if h == 1:
                        for hh in range(H_LOC):
                            wp_t = wpool.tile([128, D_MODEL], F8, tag="wp_t", bufs=4)
                            nc.sync.dma_start(wp_t[:], wp_d[hh])
                            wp_tiles.append(wp_t)

            # ================= phase B: per-head attention ====================
            with (
                tc.tile_pool(name="psS", bufs=2, space="PSUM") as psS,
                tc.tile_pool(name="psP", bufs=2, space="PSUM") as psP,
            ):
                # projection partial accumulated across heads in SBUF
                out_sb = cpool.tile([128, D_MODEL // 128], F32, tag="out_sb")
                nc.vector.memset(out_sb[:], 0.0)
                for h in range(H_LOC):
                    kt8, vt8 = cache_tiles[h]
                    ktl, vtl = ktl_tiles[h], vtl_tiles[h]
                    wp_t = wp_tiles[h]

                    # last-chunk scales: host values with the AllReduce result
                    # predicated over the page-511 rows
                    kst_last = spool.tile([128, 1], F32, tag="kst_last", bufs=4)
                    nc.gpsimd.tensor_copy(out=kst_last[:], in_=kvs[:, 2 * h : 2 * h + 1])
                    nc.vector.copy_predicated(
                        out=kst_last[:], mask=mask_tail[:], data=sb_bc[:, 4:5]
                    )
                    vst_last = spool.tile([128, 1], F32, tag="vst_last", bufs=4)
                    nc.gpsimd.tensor_copy(
                        out=vst_last[:], in_=kvs[:, 2 * h + 1 : 2 * h + 2]
                    )
                    nc.vector.copy_predicated(
                        out=vst_last[:], mask=mask_tail[:], data=sb_bc[:, 1:2]
                    )
                    # row 127 (the new token) is handled by a separate rank-1
                    # correction matmul; zero its V-path weight here
                    nc.vector.copy_predicated(
                        out=vst_last[:], mask=mask_127[:], data=zero_col[:]
                    )
                    # insert quantized new-token k into the last K chunk
                    nc.vector.tensor_copy(
                        out=ktl[:, 127:128], in_=k_ins16[:, h : h + 1]
                    )

                    ps_s = psS.tile([128, N_MAIN], F32, tag="ps_s")
                    ps_av = psS.tile([128, 1], F32, tag="ps_av")
                    # single-shot bank: col0=score63 raw, col1=score63 scaled,
                    # col2=aV c63, [0,3]=q.k_new, col4=new-token corr,
                    # [0,5]=denom, col6=A_SCALE/denom bcast
                    ps_b = psS.tile([128, 8], F32, tag="ps_b")
                    attn8 = apool.tile([128, N_MAIN], F8, tag="attn8")
                    attn63 = spool.tile([128, 1], F16, tag="attn63", bufs=4)
                    e63f = spool.tile([128, 1], F32, tag="e63f", bufs=4)
                    rowsums = spool.tile(
                        [128, len(PIECES) + 1], F32, tag="rowsums", bufs=4
                    )

                    # scores (main chunks: scales pre-folded on host)
                    for c in range(N_MAIN):
                        nc.tensor.matmul(
                            ps_s[:, c : c + 1],
                            kt8[:, 128 * c : 128 * (c + 1)],
                            q8[:, h : h + 1],
                            start=True, stop=True,
                        )
                    # last chunk: raw int8 values in fp16 (exact), fp16 q;
                    # scale applied via kst_last
                    nc.tensor.matmul(
                        ps_b[:, 0:1], ktl[:], qcol16[:, h : h + 1],
                        start=True, stop=True,
                    )
                    nc.vector.tensor_mul(
                        out=ps_b[:, 1:2], in0=ps_b[:, 0:1], in1=kst_last[:]
                    )
                    # exp piece-wise (lets aV chase the score stream);
                    # attention weights emitted as fp8 (fp16 for c63)
                    for p, (c0, c1) in enumerate(PIECES):
                        nc.scalar.activation(
                            attn8[:, c0:c1], ps_s[:, c0:c1],
                            mybir.ActivationFunctionType.Exp,
                            accum_out=rowsums[:, p : p + 1],
                        )
                    nc.scalar.activation(
                        e63f[:], ps_b[:, 1:2],
                        mybir.ActivationFunctionType.Exp,
                        accum_out=rowsums[:, len(PIECES) : len(PIECES) + 1],
                    )
                    nc.gpsimd.tensor_mul(
                        out=attn63[:], in0=e63f[:], in1=vst_last[:]
                    )

                    # aV (V scales pre-folded on host for main chunks)
                    for c in range(N_MAIN):
                        nc.tensor.matmul(
                            ps_av[:],
                            vt8[:, 128 * c : 128 * (c + 1)],
                            attn8[:, c : c + 1],
                            start=(c == 0), stop=(c == N_MAIN - 1),
                        )
                    nc.tensor.matmul(
                        ps_b[:, 2:3], vtl[:], attn63[:], start=True, stop=True
                    )
                    # new-token V contribution: w * v_i8 with
                    # w = exp(q . k_new * ksc/sqrt(dh)) * vsc
                    nc.tensor.matmul(
                        ps_b[0:1, 3:4], qcol16[:, h : h + 1],
                        k_ins16[:, h : h + 1], start=True, stop=True,
                    )
                    w_sb = spool.tile([1, 2], F32, tag="w_sb", bufs=4)
                    nc.vector.tensor_scalar_mul(
                        w_sb[:, 0:1], ps_b[0:1, 3:4], scal[0:1, 4:5]
                    )
                    nc.scalar.activation(
                        w_sb[:, 0:1], w_sb[:, 0:1],
                        mybir.ActivationFunctionType.Exp,
                    )
                    nc.vector.tensor_scalar_mul(
                        w_sb[:, 1:2], w_sb[:, 0:1], scal[0:1, 1:2]
                    )
                    w16 = spool.tile([1, 1], F16, tag="w16", bufs=4)
                    nc.vector.tensor_copy(out=w16[:], in_=w_sb[:, 1:2])
                    nc.tensor.matmul(
                        ps_b[:, 4:5], v_ins[0:1, 128 * h : 128 * (h + 1)],
                        w16[:], start=True, stop=True,
                    )

                    # denominator + A_SCALE/denom broadcast
                    rs1 = spool.tile([128, 1], F32, tag="rs1", bufs=4)
                    nc.vector.reduce_sum(rs1[:], rowsums[:], axis=mybir.AxisListType.X)
                    nc.tensor.matmul(
                        ps_b[0:1, 5:6], rs1[:], ones_col[:], start=True, stop=True
                    )
                    inv_sb = spool.tile([1, 1], F32, tag="inv_sb", bufs=4)
                    nc.vector.reciprocal(inv_sb[:], ps_b[0:1, 5:6])
                    nc.tensor.matmul(
                        ps_b[:, 6:7], r256_row[:], inv_sb[:], start=True, stop=True
                    )
                    invbc = spool.tile([128, 1], F32, tag="invbc", bufs=4)
                    nc.vector.tensor_copy(out=invbc[:], in_=ps_b[:, 6:7])
                    # head output: (aV_main + aV_c63 + corr) * A_SCALE / denom
                    # (aV accumulator via SBUF: tensor_tensor can't take two
                    # PSUM operands)
                    av_main = spool.tile([128, 1], F32, tag="av_main", bufs=4)
                    nc.scalar.copy(out=av_main[:], in_=ps_av[:])
                    av_sum = spool.tile([128, 2], F32, tag="av_sum", bufs=4)
                    nc.vector.scalar_tensor_tensor(
                        out=av_sum[:, 0:1], in0=ps_b[:, 2:3], scalar=1.0,
                        in1=av_main[:],
                        op0=mybir.AluOpType.mult, op1=mybir.AluOpType.add,
                    )
                    nc.vector.scalar_tensor_tensor(
                        out=av_sum[:, 1:2], in0=ps_b[:, 4:5], scalar=1.0,
                        in1=av_sum[:, 0:1],
                        op0=mybir.AluOpType.mult, op1=mybir.AluOpType.add,
                    )
                    nc.vector.tensor_scalar_mul(
                        a8[:, h : h + 1], av_sum[:, 1:2], invbc[:, 0:1]
                    )

                    # fold this head into the output projection (column form:
                    # ps_oc[:, oc] = this head's contribution to o = 128*oc+p,
                    # accumulated across heads in SBUF)
                    ps_oc = psP.tile([128, D_MODEL // 128], F32, tag="ps_oc")
                    for oc in range(D_MODEL // 128):
                        nc.tensor.matmul(
                            ps_oc[:, oc : oc + 1],
                            wp_t[:, 128 * oc : 128 * (oc + 1)],
                            a8[:, h : h + 1],
                            start=True, stop=True,
                        )
                    nc.vector.tensor_add(
                        out=out_sb[:], in0=ps_oc[:], in1=out_sb[:]
                    )

                if debug_out:
                    dbg = cpool.tile([128, 64], F32, tag="dbg")
                    nc.vector.memset(dbg[:], 0.0)
                    nc.vector.tensor_copy(out=dbg[:, 0:4], in_=a8[:])
                    nc.vector.tensor_copy(out=dbg[:, 4:8], in_=q8[:])
                    nc.vector.tensor_copy(out=dbg[:, 8:16], in_=sb_bc[:])
                    nc.vector.tensor_copy(out=dbg[:, 16:20], in_=k_ins16[:])
                    nc.vector.tensor_copy(out=dbg[:, 20:25], in_=rowsums[:])
                    nc.vector.tensor_copy(out=dbg[:, 25:26], in_=rs1[:])
                    nc.vector.tensor_copy(out=dbg[:, 26:28], in_=av_sum[:])
                    nc.vector.tensor_copy(out=dbg[:, 28:32], in_=qcol16[:])
                    nc.vector.tensor_copy(out=dbg[:, 32:48], in_=attn8[:, 0:16])
                    nc.sync.dma_start(dbg_d[:], dbg[:])

                # ============== phase C: store projection partial =============
                nc.sync.dma_start(out_d[:], out_sb[:])

    nc.compile()
    return nc


def prep_inputs(x, Wqkv, Wproj, K_cache, V_cache, K_scale, V_scale, page_table,
                seqlen, page_size):
    """Shard + lay out the full inputs into 8 per-core in_maps."""
    x = np.asarray(x, dtype=np.float32).reshape(-1)  # [4096]
    Wqkv = np.asarray(Wqkv, dtype=np.float32)
    Wproj = np.asarray(Wproj, dtype=np.float32)
    K_cache = np.asarray(K_cache)  # [1024, 16, 32, 128] int8
    V_cache = np.asarray(V_cache)
    K_scale = np.asarray(K_scale)  # [1024, 1, 32, 1] fp16
    V_scale = np.asarray(V_scale)
    page_table = np.asarray(page_table).astype(np.int64)  # [512]

    # x padded for DoubleRow stationary APs: cols 16j of [128, 0:512]
    xw = np.zeros((128, N_CI, 16), dtype=NP_F8)
    xw[:, :, 0] = x.reshape(N_CI, 128).T.astype(NP_F8)
    xw = xw.reshape(128, 512)

    # gather active pages once (host-side sharding step)
    Kp = K_cache[page_table]  # [512, 16, 32, 128] int8
    Vp = V_cache[page_table]
    ks = K_scale[page_table][:, 0, :, 0].astype(np.float32)  # [512, 32]
    vs = V_scale[page_table][:, 0, :, 0].astype(np.float32)

    # fold dequant scales into the cache values (fp32 -> fp8)
    Kf = Kp.astype(np.float32) * (ks * INV_SQRT_DH)[:, None, :, None]
    Vf = Vp.astype(np.float32) * vs[:, None, :, None]

    in_maps = []
    for m in range(N_CORES):
        heads = slice(H_LOC * m, H_LOC * (m + 1))
        rk = slice(D_MODEL + 512 * m, D_MODEL + 512 * (m + 1))
        rv = slice(2 * D_MODEL + 512 * m, 2 * D_MODEL + 512 * (m + 1))
        rq = slice(512 * m, 512 * (m + 1))
        # [128(p), 32(j), n]: row d = 128 j + p of the weight slice.
        # Weights ship as fp8(64*W) — see build_bass docstring.
        wq = np.ascontiguousarray(
            (Wqkv[rq].T * 64.0).reshape(N_CI, 128, 512)
            .transpose(1, 0, 2).astype(NP_F8)
        )
        wkv = np.ascontiguousarray(
            (np.concatenate([Wqkv[rk], Wqkv[rv]], axis=0).T * 64.0)
            .reshape(N_CI, 128, 1024).transpose(1, 0, 2).astype(NP_F8)
        )
        # wp[h, d, o] = 64 * Wproj[o, 512 m + 128 h + d]
        wp = np.ascontiguousarray(
            (Wproj[:, 512 * m : 512 * (m + 1)].T * 64.0)
            .reshape(H_LOC, 128, D_MODEL).astype(NP_F8)
        )

        # K.T folded per head: [4, 128(dh), 8064(l)]
        kt = np.ascontiguousarray(
            Kf[:, :, heads, :].reshape(KV_LEN, H_LOC, 128)[:L_MAIN]
            .transpose(1, 2, 0).astype(NP_F8)
        )
        # V folded, chunk layout: [4, 128(l%128), 63*128 (chunk-major dh)]
        vt = np.ascontiguousarray(
            Vf[:, :, heads, :].reshape(KV_LEN, H_LOC, 128)[:L_MAIN]
            .reshape(N_MAIN, 128, H_LOC, 128)
            .transpose(2, 1, 0, 3).reshape(H_LOC, 128, L_MAIN).astype(NP_F8)
        )
        # last chunk: raw int8 values fp16-exact (scales applied on-chip),
        # V and K packed into one DMA
        Kl = Kp[-8:, :, heads, :].reshape(128, H_LOC, 128)  # [l, h, d]
        Vl = Vp[-8:, :, heads, :].reshape(128, H_LOC, 128)
        vtl = np.concatenate(
            [
                Vl.reshape(128, 512).astype(np.float16),
                Kl.transpose(2, 1, 0).reshape(128, 512).astype(np.float16),
            ],
            axis=1,
        )
        # last-chunk scale columns [128, 2h]=K*inv_sqrt_dh, [128, 2h+1]=V
        pidx = PAGES_USED - 8 + np.arange(128) // PAGE_SIZE
        kvsc = np.empty((128, 2 * H_LOC), dtype=np.float32)
        kvsc[:, 0::2] = ks[pidx][:, heads] * INV_SQRT_DH
        kvsc[:, 1::2] = vs[pidx][:, heads]

        in_maps.append(
            dict(xk8=xw, wq=wq, wkv=wkv, wp=wp, kt=kt, vt=vt,
                 vtl=vtl, kvs=kvsc)
        )
    return in_maps, x


_NC_CACHE = None


def get_nc():
    global _NC_CACHE
    if _NC_CACHE is None:
        _NC_CACHE = build_bass()
    return _NC_CACHE


def kernel(**inputs) -> np.ndarray:
    nc = get_nc()
    in_maps, x_f32 = prep_inputs(**inputs)
    res = run_bass_kernel_spmd(nc, in_maps, list(range(N_CORES)))
    total = np.zeros(D_MODEL, dtype=np.float32)
    for c in range(N_CORES):
        # column-proj layout: out[128*oc + p] = dram[p, oc]
        total += res.results[c]["out"].T.reshape(-1)
    # wp ships as fp8(64*W) and a8 as A_SCALE*a — divide both back out
    out = x_f32 + total / (A_SCALE * 64.0)
    return out.reshape(1, 1, D_MODEL).astype(np.float32)
